# revision 51
# baseline (speedup 1.0000x reference)
"""Trainium2 Bass kernel for Mixtral-style attention (B=2, S=2048, 32 q / 8 kv heads, D=128).

Sharding: 2-way data parallel over batch x 4-way tensor parallel over heads
(8 cores). Each core computes QKV projection for its head shard, RoPE, causal
GQA attention, and a partial o_proj (row-sharded). Host sums the 4 bf16
partials per batch element in fp32.

All heavy matmuls run in bf16 with fp32 PSUM accumulation. Attention scores
are computed directly transposed (kT_blk^T @ qT_chunk) so exp(PSUM)->SBUF
lands straight in the probsT layout the attnT matmul needs; the causal mask
is a transposed-tril multiply on the diagonal 128x128 block only.

Softmax denominator: probsT blocks are group-summed on the DVE (bf16),
then ONE all-ones-stationary matmul per (head, chunk) turns the [128,512]
block-sum into the column-sum replicated across all 128 partitions. A fast
DVE reciprocal of that [128,512] tile feeds the attnT normalization multiply
directly -- no [1,512] row, no broadcast matmul.

Phase A (QKV projection, PE-bound) and phase B (attention, ACT/exp-heavy)
are software-interleaved per 512-token chunk level: while the PE chews
chunk m+1's projection matmuls, the ACT engine computes chunk m's exps and
the DVE its denominators, so no engine serializes the other. q chunks
rotate through a 2-slot buffer (chunk m is consumed by level m only).
Phase B itself keeps a 3-stage pipeline (scores(k) | den+attnV(k-1) |
epilogue(k-2)); diagonal-mask muls are emitted after the previous chunk's
den tree to avoid DVE head-of-line blocking.
"""

import os
import sys

import numpy as np

for _p in ("/opt/trn_rl_repo", "/root/.axon_site/_ro/trn_rl_repo"):
    if os.path.isdir(_p) and _p not in sys.path:
        sys.path.insert(0, _p)

import ml_dtypes  # noqa: E402

import concourse.bass as bass  # noqa: E402
import concourse.mybir as mybir  # noqa: E402
import concourse.tile as tile  # noqa: E402
from concourse import bacc, bass_utils  # noqa: E402

BF16 = ml_dtypes.bfloat16
F32 = mybir.dt.float32
BF = mybir.dt.bfloat16

B, S, HIDDEN = 2, 2048, 4096
NH, NKV, D = 32, 8, 128
TP, DP = 4, 2  # head-parallel x batch-parallel = 8 cores
QH = NH // TP  # 8 q heads per core
KH = NKV // TP  # 2 kv heads per core
NC_TILES = QH + 2 * KH  # 12 c-tiles of 128 per core (q..., k..., v...)
SC = 512  # s-chunk for phase A / attnT free dim
NSC = S // SC  # 4
NBLK = S // 128  # 16
ROPE_THETA = 10000.0
SM_SCALE = float(D) ** -0.5


def _interleave(a_list, b_list):
    """Merge two unit lists evenly (a spread across b)."""
    out = []
    ai = bi = 0
    na, nb = len(a_list), len(b_list)
    while ai < na or bi < nb:
        if bi >= nb or (ai < na and ai * nb <= bi * na):
            out.append(a_list[ai])
            ai += 1
        else:
            out.append(b_list[bi])
            bi += 1
    return out


def _emit(nc: bass.Bass):
    hT = nc.dram_tensor("hT", [128, HIDDEN // 128, S], BF, kind="ExternalInput")
    wq = nc.dram_tensor("wq", [NC_TILES, 128, 32 * 128], BF, kind="ExternalInput")
    wo = nc.dram_tensor("wo", [8, 128, 8 * 512], BF, kind="ExternalInput")
    cosT = nc.dram_tensor("cosT", [128, S], BF, kind="ExternalInput")
    sinT = nc.dram_tensor("sinT", [128, S], BF, kind="ExternalInput")
    triuD = nc.dram_tensor("triuD", [128, 128], BF, kind="ExternalInput")
    onesD = nc.dram_tensor("onesD", [1, 128], BF, kind="ExternalInput")
    onesMD = nc.dram_tensor("onesMD", [128, 128], BF, kind="ExternalInput")
    out = nc.dram_tensor("out", [S, HIDDEN], BF, kind="ExternalOutput")

    with tile.TileContext(nc) as tc:
        with (
            tc.tile_pool(name="const", bufs=1) as constp,
            tc.tile_pool(name="big", bufs=2) as bigp,
            tc.tile_pool(name="slab", bufs=2) as slabp,
            tc.tile_pool(name="wt", bufs=3) as wtp,
            tc.tile_pool(name="pers", bufs=1) as pers,
            tc.tile_pool(name="rope", bufs=1) as ropep,
            tc.tile_pool(name="acc", bufs=2) as accp,
            tc.tile_pool(name="rcp", bufs=1) as rcpp,
            tc.tile_pool(name="outp", bufs=4) as outp,
            tc.tile_pool(name="psum", bufs=2, space="PSUM") as psum,
            tc.tile_pool(name="psum_s", bufs=4, space="PSUM") as psum_s,
        ):
            # tiny consts first so the PE warm-up can start immediately
            triu = constp.tile([128, 128], BF, tag="triu")
            ones1 = constp.tile([1, 128], BF, tag="ones1")
            onesM = constp.tile([128, 128], BF, tag="onesM")
            nc.sync.dma_start(ones1, onesD[:])
            nc.sync.dma_start(triu, triuD[:])
            nc.sync.dma_start(onesM, onesMD[:])

            cos_sb = constp.tile([128, S], BF, tag="cos")
            sin_sb = constp.tile([128, S], BF, tag="sin")

            # persistent activations; q chunks rotate through 2 slots
            qT = pers.tile([128, QH, 2, SC], BF, tag="qT")  # [d, head, slot, s]
            kT = pers.tile([128, KH, S], BF, tag="kT")
            vN = pers.tile([128, KH * NBLK, 128], BF, tag="vN")  # [sk, kv*blk, d]
            aT = pers.tile([128, QH, S], BF, tag="aT")  # [d, head, s]

            def rope_into(dst, ps, sc):
                # dst = ps * cos + rot(ps) * sin ; rot = [-x2, x1]
                rot = ropep.tile([128, SC], F32, tag="rot")
                nc.scalar.mul(rot[0:64, :], ps[64:128, :], -1.0)
                nc.scalar.copy(rot[64:128, :], ps[0:64, :])
                t2 = ropep.tile([128, SC], F32, tag="t2")
                cs = cos_sb[:, sc * SC : (sc + 1) * SC]
                sn = sin_sb[:, sc * SC : (sc + 1) * SC]
                nc.vector.tensor_mul(t2, ps, cs)
                nc.vector.tensor_mul(rot, rot, sn)
                nc.vector.tensor_add(dst, t2, rot)

            # ---- Phase A units: one c-tile of QKV^T = w^T @ h^T + RoPE ----
            def emit_hTc(sc, parts=16):
                t = bigp.tile([128, 32, SC], BF, tag="hTc")
                w = 32 // parts
                for hq in range(parts):
                    nc.sync.dma_start(
                        t[:, hq * w : (hq + 1) * w, :],
                        hT[:, hq * w : (hq + 1) * w, sc * SC : (sc + 1) * SC],
                    )
                return t

            def emit_wct(c, parts=8):
                wct = wtp.tile([128, 32 * 128], BF, tag="wt")
                w = 4096 // parts
                for hq in range(parts):
                    nc.sync.dma_start(
                        wct[:, hq * w : (hq + 1) * w],
                        wq[c, :, hq * w : (hq + 1) * w],
                    )
                return wct

            def a_unit(sc, c, hTc, wct=None):
                if wct is None:
                    # finer splits in the prologue: queues carry the initial
                    # burst and per-chunk latency is the stall driver there
                    wct = emit_wct(c, parts=16 if sc == 0 else 8)
                ps = psum.tile([128, SC], F32, tag="mm512")
                for ho in range(32):
                    nc.tensor.matmul(
                        ps,
                        wct[:, ho * 128 : (ho + 1) * 128],
                        hTc[:, ho, :],
                        start=(ho == 0),
                        stop=(ho == 31),
                        skip_group_check=True,
                    )
                if c < QH:
                    rope_into(qT[:, c, sc % 2, :], ps, sc)
                elif c < QH + KH:
                    rope_into(kT[:, c - QH, sc * SC : (sc + 1) * SC], ps, sc)
                else:
                    kv = c - QH - KH
                    vt = ropep.tile([128, SC], BF, tag="vt")
                    nc.scalar.copy(vt, ps)
                    for j in range(SC // 128):
                        blk = sc * 4 + j
                        nc.sync.dma_start(
                            vN[:, kv * NBLK + blk, :],
                            vt[:, j * 128 : (j + 1) * 128],
                            transpose=True,
                        )

            # ---- Phase B units ----
            # slab[:, j, :] holds (unnormalized) probsT for sk-block j of the
            # current sq-chunk: all scores are computed directly transposed
            # (kT_blk^T @ qT_chunk) + exp from PSUM. Diagonal rows only cover
            # their causal sq columns; the diagonal 128x128 block gets a
            # transposed-tril (triu) mask applied post-exp (emitted late, see
            # b_unit).
            def b_scores(h, m):
                kv = h // (QH // KH)
                slab = slabp.tile([128, NBLK, SC], BF, tag="slab")
                qm = qT[:, h, m % 2, :]
                for j in range(4 * m + 4):
                    jj = j - 4 * m  # >= 0 for diagonal-region rows
                    c0 = max(0, jj) * 128
                    sps = psum_s.tile([128, 512], F32, tag="scores")
                    nc.tensor.matmul(
                        sps[:, : 512 - c0],
                        kT[:, kv, j * 128 : (j + 1) * 128],
                        qm[:, c0:],
                        start=True,
                        stop=True,
                        skip_group_check=True,
                    )
                    nc.scalar.activation(
                        slab[:, j, c0:],
                        sps[:, : 512 - c0],
                        mybir.ActivationFunctionType.Exp,
                        scale=SM_SCALE,
                    )
                return slab

            def b_mask(m, slab):
                for jj in range(4):
                    j = 4 * m + jj
                    c0 = jj * 128
                    blk = slab[:, j, c0 : c0 + 128]
                    nc.vector.tensor_mul(blk, blk, triu)

            def b_denattn(h, m, slab):
                kv = h // (QH // KH)
                # DVE block-sum of slab -> acc[:, 0, :]. Diagonal group (last
                # four j blocks) is width-aware in slot 1; full blocks chain
                # into slot 0. All bf16 SBUF (fast DVE mode).
                acc = accp.tile([128, 2, 512], BF, tag="acc")
                sd = 1 if m > 0 else 0
                nc.vector.tensor_copy(acc[:, sd, :], slab[:, 4 * m, :])
                for jj in range(1, 4):
                    c0 = jj * 128
                    nc.vector.tensor_add(
                        acc[:, sd, c0:], acc[:, sd, c0:], slab[:, 4 * m + jj, c0:]
                    )
                if m > 0:
                    nc.vector.tensor_add(acc[:, 0, :], slab[:, 0, :], slab[:, 1, :])
                    for j in range(2, 4 * m):
                        nc.vector.tensor_add(acc[:, 0, :], acc[:, 0, :], slab[:, j, :])
                    nc.vector.tensor_add(acc[:, 0, :], acc[:, 0, :], acc[:, 1, :])

                # attnV: accumulate over all j blocks
                aps = psum.tile([128, 512], F32, tag="attn")
                for j in range(4 * m):
                    nc.tensor.matmul(
                        aps, vN[:, kv * NBLK + j, :], slab[:, j, :],
                        start=(j == 0), stop=False, skip_group_check=True,
                    )
                for jj in range(4):
                    j = 4 * m + jj
                    cs = slice(jj * 128, 512)
                    first = m == 0 and jj == 0
                    nc.tensor.matmul(
                        aps[:, cs], vN[:, kv * NBLK + j, :], slab[:, j, cs],
                        start=first, stop=(jj == 3), skip_group_check=True,
                    )
                # den replicated to all partitions: ones[128,128]^T @ acc0
                den = psum.tile([128, 512], F32, tag="mm512")
                nc.tensor.matmul(
                    den, onesM, acc[:, 0, :], start=True, stop=True,
                    skip_group_check=True,
                )
                rcp = rcpp.tile([128, 512], F32, tag="rcp")
                nc.vector.reciprocal_approx_fast(rcp, den)
                return aps, rcp

            def b_epilogue(h, m, aps, rcp):
                nc.vector.tensor_mul(aT[:, h, m * 512 : (m + 1) * 512], aps, rcp)

            # 3-stage pipeline state over B units (global across levels)
            bstate = {"s1": None, "s2": None}

            def b_unit(h, m):
                slab = b_scores(h, m)
                s1, s2 = bstate["s1"], bstate["s2"]
                # epilogue(k-2) BEFORE denattn(k-1): keeps the single rcp
                # buffer strictly read-then-write in DVE program order
                if s1 is not None and s2 is not None:
                    b_epilogue(*s2)
                if s1 is not None:
                    ph, pm, pslab = s1
                    bstate["s2"] = (ph, pm) + b_denattn(ph, pm, pslab)
                b_mask(m, slab)  # after prev chunk's den tree (DVE order)
                bstate["s1"] = (h, m, slab)

            def b_flush():
                s1, s2 = bstate["s1"], bstate["s2"]
                if s2 is not None:
                    b_epilogue(*s2)
                ph, pm, pslab = s1
                b_epilogue(ph, pm, *b_denattn(ph, pm, pslab))

            # ---- Prologue: first c-tile inputs in flight during warm-up ----
            # ultra-fine splits so the first chunks land on idle queues fast;
            # weights first -- the first LDWEIGHTS is the startup blocker
            wct0 = emit_wct(0, parts=16)
            hcur = emit_hTc(0, parts=32)

            # HAM warm-up: ~10us of dummy matmuls on the tiny constants
            # while the first hidden/weight DMAs are in flight, so the PE
            # is already un-throttled (K=8/8) when real data arrives.
            wps = psum_s.tile([128, 512], F32, tag="scores")
            for w in range(48):
                nc.tensor.matmul(
                    wps[:, :128], ones1, ones1, start=(w == 0), stop=(w == 47),
                    skip_group_check=True,
                )
            dwarm = rcpp.tile([128, 512], F32, tag="rcp")
            nc.vector.tensor_copy(dwarm[:, :128], wps[:, :128])

            nc.sync.dma_start(cos_sb, cosT[:])
            nc.sync.dma_start(sin_sb, sinT[:])
            a_unit(0, 0, hcur, wct0)
            for c in range(1, NC_TILES):
                a_unit(0, c, hcur)
                if c == 9:
                    b_unit(0, 0)  # spread early exps into the prologue
                elif c == 10:
                    b_unit(1, 0)

            # ---- Phase C helpers: o_proj st-tiles for one w_o head-tile ----
            wo_tiles = {}

            def c_prefetch(hc):
                wot = wtp.tile([128, 8 * 512], BF, tag="wt")
                for hq in range(8):
                    nc.sync.dma_start(
                        wot[:, hq * 512 : (hq + 1) * 512],
                        wo[hc, :, hq * 512 : (hq + 1) * 512],
                    )
                wo_tiles[hc] = wot

            def c_tiles(hc, sts, pf=None, deep=False, act_copy=False):
                wot = wo_tiles[hc]
                for sti, st in enumerate(sts):
                    if pf is not None and sti == len(sts) // 2:
                        c_prefetch(pf)
                    # after phase B both psum pools are free: alternate tags
                    # for a 4-deep rotation so the DVE cast is never waited on
                    tag = ("attn" if sti % 2 else "mm512") if deep else "mm512"
                    ops = psum.tile([128, 512], F32, tag=tag)
                    for cb in range(QH):
                        nc.tensor.matmul(
                            ops,
                            aT[:, cb, st * 128 : (st + 1) * 128],
                            wot[:, cb * 512 : (cb + 1) * 512],
                            start=(cb == 0),
                            stop=(cb == QH - 1),
                            skip_group_check=True,
                        )
                    ot = outp.tile([128, 512], BF, tag="ot")
                    if act_copy:  # DVE may still be draining the flush chain
                        nc.scalar.copy(ot, ops)
                    else:
                        nc.vector.tensor_copy(ot, ops)
                    for half in range(2):
                        nc.sync.dma_start(
                            out[
                                st * 128 : (st + 1) * 128,
                                hc * 512 + half * 256 : hc * 512 + (half + 1) * 256,
                            ],
                            ot[:, half * 256 : (half + 1) * 256],
                        )

            # ---- Levels: A(m+1) interleaved with B(*, m); level 3 weaves
            # in early o_proj tiles for hc=0,1 (aT chunks 0..2 are ready) ----
            for m in range(NSC):
                h0 = 2 if m == 0 else 0  # first two m=0 units are in prologue
                bunits = [(lambda h=h, m=m: b_unit(h, m)) for h in range(h0, QH)]
                if m < NSC - 1:
                    hnext = emit_hTc(m + 1)
                    aunits = [
                        (lambda sc=m + 1, c=c, t=hnext: a_unit(sc, c, t))
                        for c in range(NC_TILES)
                    ]
                    # lead with B units so the fresh hTc DMA gets headroom
                    lead = 2 if m == 0 else 1
                    merged = bunits[:lead] + _interleave(aunits, bunits[lead:])
                else:
                    c_prefetch(0)
                    c_prefetch(1)
                    fillers = [
                        (lambda hc=hc, g=g: c_tiles(hc, range(4 * g, 4 * g + 4)))
                        for g in range(3)
                        for hc in (0, 1)
                    ]
                    merged = bunits[:2] + _interleave(fillers, bunits[2:])
                for u in merged:
                    u()
            c_prefetch(2)
            # flush with PE cover: independent o_proj tiles run while the
            # final den/recip/epilogue DVE chain drains
            fs1, fs2 = bstate["s1"], bstate["s2"]
            if fs2 is not None:
                b_epilogue(*fs2)
            fph, fpm, fpslab = fs1
            fres = b_denattn(fph, fpm, fpslab)
            c_tiles(2, range(0, 4), act_copy=True)
            b_epilogue(fph, fpm, *fres)

            # ---- Phase C remainder ----
            c_tiles(0, range(12, 16), pf=3, deep=True, act_copy=True)
            c_tiles(1, range(12, 16), pf=4, deep=True, act_copy=True)
            c_tiles(2, range(4, 16), pf=5, deep=True)
            for hc in range(3, 8):
                c_tiles(hc, range(16), pf=hc + 3 if hc + 3 <= 7 else None,
                        deep=True)

    return nc


_CACHE = {}


def build_program():
    if "nc" not in _CACHE:
        nc = bacc.Bacc()
        _emit(nc)
        nc.compile()
        _CACHE["nc"] = nc
    return _CACHE["nc"]


def host_inputs(positions, hidden_states, w_qkv, w_o):
    """Build the 8 per-core input maps (host-side shard + layout + bf16 cast)."""
    positions = np.asarray(positions)
    hidden_states = np.asarray(hidden_states, dtype=np.float32)
    w_qkv = np.asarray(w_qkv, dtype=np.float32)
    w_o = np.asarray(w_o, dtype=np.float32)

    inv_freq = 1.0 / (
        ROPE_THETA ** (np.arange(0, D, 2, dtype=np.float32) / D)
    )  # [64]
    trium = np.triu(np.ones((128, 128), dtype=np.float32)).astype(BF16)

    # per-batch tensors
    hTs, coss, sins = [], [], []
    for b in range(B):
        hT = (
            np.ascontiguousarray(hidden_states[b].T)  # [HIDDEN, S]
            .reshape(HIDDEN // 128, 128, S)
            .transpose(1, 0, 2)  # [128, ho, S]
        )
        hTs.append(np.ascontiguousarray(hT.astype(BF16)))
        ang = positions[b].astype(np.float32)[:, None] * inv_freq[None, :]  # [S,64]
        c = np.cos(ang).T  # [64, S]
        s = np.sin(ang).T
        coss.append(np.concatenate([c, c], axis=0).astype(BF16))
        sins.append(np.concatenate([s, s], axis=0).astype(BF16))

    in_maps = []
    for core in range(8):
        b, t = divmod(core, TP)
        qcols = w_qkv[:, t * QH * D : (t + 1) * QH * D]
        kcols = w_qkv[:, NH * D + t * KH * D : NH * D + (t + 1) * KH * D]
        vcols = w_qkv[:, (NH + NKV) * D + t * KH * D : (NH + NKV) * D + (t + 1) * KH * D]
        wshard = np.concatenate([qcols, kcols, vcols], axis=1)  # [4096, 1536]
        wq_t = (
            wshard.reshape(32, 128, NC_TILES, 128)
            .transpose(2, 1, 0, 3)  # [c, p, ho, m]
            .reshape(NC_TILES, 128, 32 * 128)
            .astype(BF16)
        )
        wo_shard = w_o[t * QH * D : (t + 1) * QH * D, :]  # [1024, 4096]
        wo_t = (
            wo_shard.reshape(QH, 128, 8, 512)
            .transpose(2, 1, 0, 3)  # [hc, p, co, n]
            .reshape(8, 128, 8 * 512)
            .astype(BF16)
        )
        in_maps.append(
            {
                "hT": hTs[b],
                "wq": np.ascontiguousarray(wq_t),
                "wo": np.ascontiguousarray(wo_t),
                "cosT": coss[b],
                "sinT": sins[b],
                "triuD": trium,
                "onesD": np.ones((1, 128), dtype=BF16),
                "onesMD": np.ones((128, 128), dtype=BF16),
            }
        )
    return in_maps


def gather_output(results):
    """Sum the 4 TP bf16 partials per batch in fp32 -> [B, S, HIDDEN]."""
    outs = []
    for b in range(B):
        acc = np.zeros((S, HIDDEN), dtype=np.float32)
        for t in range(TP):
            acc += results[b * TP + t]["out"].astype(np.float32)
        outs.append(acc)
    return np.stack(outs, axis=0)


def kernel(positions, hidden_states, w_qkv, w_o, trace=False):
    nc = build_program()
    in_maps = host_inputs(positions, hidden_states, w_qkv, w_o)
    last_err = None
    for attempt in range(3):
        try:
            res = bass_utils.run_bass_kernel_spmd(
                nc, in_maps, core_ids=list(range(8)), trace=trace
            )
            break
        except Exception as e:  # transient NRT/axon device errors
            last_err = e
            import time as _time

            _time.sleep(5 * (attempt + 1))
    else:
        raise last_err
    out = gather_output(res.results)
    if trace:
        kernel.last_exec_time_ns = res.exec_time_ns
        kernel.last_results = res
    return out


# revision 57
# speedup vs baseline: 1.1542x; 1.1542x over previous
"""Trainium2 Bass kernel for Mixtral-style attention (B=2, S=2048, 32 q / 8 kv heads, D=128).

Sharding: 2-way data parallel over batch x 4-way tensor parallel over heads
(8 cores). Each core computes QKV projection for its head shard, RoPE, causal
GQA attention, and a partial o_proj (row-sharded). Host sums the 4 bf16
partials per batch element in fp32.

All heavy matmuls run in bf16 with fp32 PSUM accumulation. Attention scores
are computed directly transposed (kT_blk^T @ qT_chunk) so exp(PSUM)->SBUF
lands straight in the probsT layout the attnT matmul needs; the causal mask
is a transposed-tril multiply on the diagonal 128x128 block only.

Softmax denominator: probsT blocks are group-summed on the DVE (bf16),
then ONE all-ones-stationary matmul per (head, chunk) turns the [128,512]
block-sum into the column-sum replicated across all 128 partitions. A fast
DVE reciprocal of that [128,512] tile feeds the attnT normalization multiply
directly -- no [1,512] row, no broadcast matmul.

Phase A (QKV projection, PE-bound) and phase B (attention, ACT/exp-heavy)
are software-interleaved per 512-token chunk level: while the PE chews
chunk m+1's projection matmuls, the ACT engine computes chunk m's exps and
the DVE its denominators, so no engine serializes the other. q chunks
rotate through a 2-slot buffer (chunk m is consumed by level m only).
Phase B itself keeps a 3-stage pipeline (scores(k) | den+attnV(k-1) |
epilogue(k-2)); diagonal-mask muls are emitted after the previous chunk's
den tree to avoid DVE head-of-line blocking.
"""

import os
import sys

import numpy as np

for _p in ("/opt/trn_rl_repo", "/root/.axon_site/_ro/trn_rl_repo"):
    if os.path.isdir(_p) and _p not in sys.path:
        sys.path.insert(0, _p)

import ml_dtypes  # noqa: E402

import concourse.bass as bass  # noqa: E402
import concourse.mybir as mybir  # noqa: E402
import concourse.tile as tile  # noqa: E402
from concourse import bacc, bass_utils  # noqa: E402

BF16 = ml_dtypes.bfloat16
F32 = mybir.dt.float32
BF = mybir.dt.bfloat16

B, S, HIDDEN = 2, 2048, 4096
NH, NKV, D = 32, 8, 128
TP, DP = 4, 2  # head-parallel x batch-parallel = 8 cores
QH = NH // TP  # 8 q heads per core
KH = NKV // TP  # 2 kv heads per core
NC_TILES = QH + 2 * KH  # 12 c-tiles of 128 per core (q..., k..., v...)
SC = 512  # s-chunk for phase A / attnT free dim
NSC = S // SC  # 4
NBLK = S // 128  # 16
ROPE_THETA = 10000.0
SM_SCALE = float(D) ** -0.5


def _interleave(a_list, b_list):
    """Merge two unit lists evenly (a spread across b)."""
    out = []
    ai = bi = 0
    na, nb = len(a_list), len(b_list)
    while ai < na or bi < nb:
        if bi >= nb or (ai < na and ai * nb <= bi * na):
            out.append(a_list[ai])
            ai += 1
        else:
            out.append(b_list[bi])
            bi += 1
    return out


def _emit(nc: bass.Bass):
    hT = nc.dram_tensor("hT", [128, HIDDEN // 128, S], BF, kind="ExternalInput")
    wq = nc.dram_tensor("wq", [NC_TILES, 128, 32 * 128], BF, kind="ExternalInput")
    wo = nc.dram_tensor("wo", [8, 128, 8 * 512], BF, kind="ExternalInput")
    cosT = nc.dram_tensor("cosT", [128, S], BF, kind="ExternalInput")
    sinT = nc.dram_tensor("sinT", [128, S], BF, kind="ExternalInput")
    triuD = nc.dram_tensor("triuD", [128, 128], BF, kind="ExternalInput")
    onesD = nc.dram_tensor("onesD", [1, 128], BF, kind="ExternalInput")
    onesMD = nc.dram_tensor("onesMD", [128, 128], BF, kind="ExternalInput")
    out = nc.dram_tensor("out", [S, HIDDEN], BF, kind="ExternalOutput")

    with tile.TileContext(nc) as tc:
        with (
            tc.tile_pool(name="const", bufs=1) as constp,
            tc.tile_pool(name="big", bufs=2) as bigp,
            tc.tile_pool(name="slab", bufs=2) as slabp,
            tc.tile_pool(name="wt", bufs=3) as wtp,
            tc.tile_pool(name="pers", bufs=1) as pers,
            tc.tile_pool(name="rope", bufs=1) as ropep,
            tc.tile_pool(name="acc", bufs=2) as accp,
            tc.tile_pool(name="rcp", bufs=1) as rcpp,
            tc.tile_pool(name="outp", bufs=4) as outp,
            tc.tile_pool(name="psum", bufs=2, space="PSUM") as psum,
            tc.tile_pool(name="psum_s", bufs=4, space="PSUM") as psum_s,
        ):
            # tiny consts first so the PE warm-up can start immediately
            triu = constp.tile([128, 128], BF, tag="triu")
            ones1 = constp.tile([1, 128], BF, tag="ones1")
            onesM = constp.tile([128, 128], BF, tag="onesM")
            nc.sync.dma_start(ones1, onesD[:])
            nc.sync.dma_start(triu, triuD[:])
            nc.sync.dma_start(onesM, onesMD[:])

            cos_sb = constp.tile([128, S], BF, tag="cos")
            sin_sb = constp.tile([128, S], BF, tag="sin")

            # persistent activations; q chunks rotate through 2 slots
            qT = pers.tile([128, QH, 2, SC], BF, tag="qT")  # [d, head, slot, s]
            kT = pers.tile([128, KH, S], BF, tag="kT")
            vN = pers.tile([128, KH * NBLK, 128], BF, tag="vN")  # [sk, kv*blk, d]
            aT = pers.tile([128, QH, S], BF, tag="aT")  # [d, head, s]

            def rope_into(dst, ps, sc):
                # dst = ps * cos + rot(ps) * sin ; rot = [-x2, x1]
                rot = ropep.tile([128, SC], F32, tag="rot")
                nc.scalar.mul(rot[0:64, :], ps[64:128, :], -1.0)
                nc.scalar.copy(rot[64:128, :], ps[0:64, :])
                t2 = ropep.tile([128, SC], F32, tag="t2")
                cs = cos_sb[:, sc * SC : (sc + 1) * SC]
                sn = sin_sb[:, sc * SC : (sc + 1) * SC]
                nc.vector.tensor_mul(t2, ps, cs)
                nc.vector.tensor_mul(rot, rot, sn)
                nc.vector.tensor_add(dst, t2, rot)

            # ---- Phase A units: one c-tile of QKV^T = w^T @ h^T + RoPE ----
            def emit_hTc(sc, parts=16):
                t = bigp.tile([128, 32, SC], BF, tag="hTc")
                w = 32 // parts
                for hq in range(parts):
                    nc.sync.dma_start(
                        t[:, hq * w : (hq + 1) * w, :],
                        hT[:, hq * w : (hq + 1) * w, sc * SC : (sc + 1) * SC],
                    )
                return t

            def emit_wct(c, parts=8):
                wct = wtp.tile([128, 32 * 128], BF, tag="wt")
                w = 4096 // parts
                for hq in range(parts):
                    nc.sync.dma_start(
                        wct[:, hq * w : (hq + 1) * w],
                        wq[c, :, hq * w : (hq + 1) * w],
                    )
                return wct

            def a_unit(sc, c, hTc, wct=None):
                if wct is None:
                    # finer splits in the prologue: queues carry the initial
                    # burst and per-chunk latency is the stall driver there
                    wct = emit_wct(c, parts=16 if sc == 0 else 8)
                ps = psum.tile([128, SC], F32, tag="mm512")
                for ho in range(32):
                    nc.tensor.matmul(
                        ps,
                        wct[:, ho * 128 : (ho + 1) * 128],
                        hTc[:, ho, :],
                        start=(ho == 0),
                        stop=(ho == 31),
                        skip_group_check=True,
                    )
                if c < QH:
                    rope_into(qT[:, c, sc % 2, :], ps, sc)
                elif c < QH + KH:
                    rope_into(kT[:, c - QH, sc * SC : (sc + 1) * SC], ps, sc)
                else:
                    kv = c - QH - KH
                    vt = ropep.tile([128, SC], BF, tag="vt")
                    nc.scalar.copy(vt, ps)
                    for j in range(SC // 128):
                        blk = sc * 4 + j
                        nc.sync.dma_start(
                            vN[:, kv * NBLK + blk, :],
                            vt[:, j * 128 : (j + 1) * 128],
                            transpose=True,
                        )

            # ---- Phase B units ----
            # slab[:, j, :] holds (unnormalized) probsT for sk-block j of the
            # current sq-chunk: all scores are computed directly transposed
            # (kT_blk^T @ qT_chunk) + exp from PSUM. Diagonal rows only cover
            # their causal sq columns; the diagonal 128x128 block gets a
            # transposed-tril (triu) mask applied post-exp (emitted late, see
            # b_unit).
            def b_scores(h, m):
                kv = h // (QH // KH)
                slab = slabp.tile([128, NBLK, SC], BF, tag="slab")
                qm = qT[:, h, m % 2, :]
                for j in range(4 * m + 4):
                    jj = j - 4 * m  # >= 0 for diagonal-region rows
                    c0 = max(0, jj) * 128
                    sps = psum_s.tile([128, 512], F32, tag="scores")
                    nc.tensor.matmul(
                        sps[:, : 512 - c0],
                        kT[:, kv, j * 128 : (j + 1) * 128],
                        qm[:, c0:],
                        start=True,
                        stop=True,
                        skip_group_check=True,
                    )
                    nc.scalar.activation(
                        slab[:, j, c0:],
                        sps[:, : 512 - c0],
                        mybir.ActivationFunctionType.Exp,
                        scale=SM_SCALE,
                    )
                return slab

            def b_mask(m, slab):
                for jj in range(4):
                    j = 4 * m + jj
                    c0 = jj * 128
                    blk = slab[:, j, c0 : c0 + 128]
                    nc.vector.tensor_mul(blk, blk, triu)

            def b_denattn(h, m, slab):
                kv = h // (QH // KH)
                # DVE block-sum of slab -> acc[:, 0, :]. Diagonal group (last
                # four j blocks) is width-aware in slot 1; full blocks chain
                # into slot 0. All bf16 SBUF (fast DVE mode).
                acc = accp.tile([128, 2, 512], BF, tag="acc")
                sd = 1 if m > 0 else 0
                nc.vector.tensor_copy(acc[:, sd, :], slab[:, 4 * m, :])
                for jj in range(1, 4):
                    c0 = jj * 128
                    nc.vector.tensor_add(
                        acc[:, sd, c0:], acc[:, sd, c0:], slab[:, 4 * m + jj, c0:]
                    )
                if m > 0:
                    nc.vector.tensor_add(acc[:, 0, :], slab[:, 0, :], slab[:, 1, :])
                    for j in range(2, 4 * m):
                        nc.vector.tensor_add(acc[:, 0, :], acc[:, 0, :], slab[:, j, :])
                    nc.vector.tensor_add(acc[:, 0, :], acc[:, 0, :], acc[:, 1, :])

                # attnV: accumulate over all j blocks
                aps = psum.tile([128, 512], F32, tag="attn")
                for j in range(4 * m):
                    nc.tensor.matmul(
                        aps, vN[:, kv * NBLK + j, :], slab[:, j, :],
                        start=(j == 0), stop=False, skip_group_check=True,
                    )
                for jj in range(4):
                    j = 4 * m + jj
                    cs = slice(jj * 128, 512)
                    first = m == 0 and jj == 0
                    nc.tensor.matmul(
                        aps[:, cs], vN[:, kv * NBLK + j, :], slab[:, j, cs],
                        start=first, stop=(jj == 3), skip_group_check=True,
                    )
                # den replicated to all partitions: ones[128,128]^T @ acc0
                den = psum.tile([128, 512], F32, tag="mm512")
                nc.tensor.matmul(
                    den, onesM, acc[:, 0, :], start=True, stop=True,
                    skip_group_check=True,
                )
                rcp = rcpp.tile([128, 512], F32, tag="rcp")
                nc.vector.reciprocal_approx_fast(rcp, den)
                return aps, rcp

            def b_epilogue(h, m, aps, rcp):
                nc.vector.tensor_mul(aT[:, h, m * 512 : (m + 1) * 512], aps, rcp)

            # 3-stage pipeline state over B units (global across levels)
            bstate = {"s1": None, "s2": None}

            def b_unit(h, m):
                slab = b_scores(h, m)
                s1, s2 = bstate["s1"], bstate["s2"]
                # epilogue(k-2) BEFORE denattn(k-1): keeps the single rcp
                # buffer strictly read-then-write in DVE program order
                if s1 is not None and s2 is not None:
                    b_epilogue(*s2)
                if s1 is not None:
                    ph, pm, pslab = s1
                    bstate["s2"] = (ph, pm) + b_denattn(ph, pm, pslab)
                b_mask(m, slab)  # after prev chunk's den tree (DVE order)
                bstate["s1"] = (h, m, slab)

            def b_flush():
                s1, s2 = bstate["s1"], bstate["s2"]
                if s2 is not None:
                    b_epilogue(*s2)
                ph, pm, pslab = s1
                b_epilogue(ph, pm, *b_denattn(ph, pm, pslab))

            # ---- Prologue: first c-tile inputs in flight during warm-up ----
            # ultra-fine splits so the first chunks land on idle queues fast;
            # weights first -- the first LDWEIGHTS is the startup blocker
            wct0 = emit_wct(0, parts=16)
            hcur = emit_hTc(0, parts=32)

            # HAM warm-up: ~10us of dummy matmuls on the tiny constants
            # while the first hidden/weight DMAs are in flight, so the PE
            # is already un-throttled (K=8/8) when real data arrives.
            wps = psum_s.tile([128, 512], F32, tag="scores")
            for w in range(48):
                nc.tensor.matmul(
                    wps[:, :128], ones1, ones1, start=(w == 0), stop=(w == 47),
                    skip_group_check=True,
                )
            dwarm = rcpp.tile([128, 512], F32, tag="rcp")
            nc.vector.tensor_copy(dwarm[:, :128], wps[:, :128])

            nc.sync.dma_start(cos_sb, cosT[:])
            nc.sync.dma_start(sin_sb, sinT[:])
            a_unit(0, 0, hcur, wct0)
            for c in range(1, NC_TILES):
                a_unit(0, c, hcur)
                if c == 9:
                    b_unit(0, 0)  # spread early exps into the prologue
                elif c == 10:
                    b_unit(1, 0)

            # ---- Phase C helpers: o_proj st-tiles for one w_o head-tile ----
            wo_tiles = {}

            def c_prefetch(hc):
                wot = wtp.tile([128, 8 * 512], BF, tag="wt")
                for hq in range(8):
                    nc.sync.dma_start(
                        wot[:, hq * 512 : (hq + 1) * 512],
                        wo[hc, :, hq * 512 : (hq + 1) * 512],
                    )
                wo_tiles[hc] = wot

            def c_tiles(hc, sts, pf=None, deep=False, act_copy=False):
                wot = wo_tiles[hc]
                for sti, st in enumerate(sts):
                    if pf is not None and sti == len(sts) // 2:
                        c_prefetch(pf)
                    # after phase B both psum pools are free: alternate tags
                    # for a 4-deep rotation so the DVE cast is never waited on
                    tag = ("attn" if sti % 2 else "mm512") if deep else "mm512"
                    ops = psum.tile([128, 512], F32, tag=tag)
                    for cb in range(QH):
                        nc.tensor.matmul(
                            ops,
                            aT[:, cb, st * 128 : (st + 1) * 128],
                            wot[:, cb * 512 : (cb + 1) * 512],
                            start=(cb == 0),
                            stop=(cb == QH - 1),
                            skip_group_check=True,
                        )
                    ot = outp.tile([128, 512], BF, tag="ot")
                    if act_copy:  # DVE may still be draining the flush chain
                        nc.scalar.copy(ot, ops)
                    else:
                        nc.vector.tensor_copy(ot, ops)
                    for half in range(2):
                        nc.sync.dma_start(
                            out[
                                st * 128 : (st + 1) * 128,
                                hc * 512 + half * 256 : hc * 512 + (half + 1) * 256,
                            ],
                            ot[:, half * 256 : (half + 1) * 256],
                        )

            # ---- Levels: A(m+1) interleaved with B(*, m); level 3 weaves
            # in early o_proj tiles for hc=0,1 (aT chunks 0..2 are ready) ----
            for m in range(NSC):
                h0 = 2 if m == 0 else 0  # first two m=0 units are in prologue
                bunits = [(lambda h=h, m=m: b_unit(h, m)) for h in range(h0, QH)]
                if m < NSC - 1:
                    hnext = emit_hTc(m + 1)
                    aunits = [
                        (lambda sc=m + 1, c=c, t=hnext: a_unit(sc, c, t))
                        for c in range(NC_TILES)
                    ]
                    # lead with B units so the fresh hTc DMA gets headroom
                    lead = 2 if m == 0 else 1
                    merged = bunits[:lead] + _interleave(aunits, bunits[lead:])
                else:
                    c_prefetch(0)
                    c_prefetch(1)
                    fillers = [
                        (lambda hc=hc, g=g: c_tiles(hc, range(4 * g, 4 * g + 4)))
                        for g in range(3)
                        for hc in (0, 1)
                    ]
                    merged = bunits[:2] + _interleave(fillers, bunits[2:])
                for u in merged:
                    u()
            c_prefetch(2)
            # flush with PE cover: independent o_proj tiles run while the
            # final den/recip/epilogue DVE chain drains
            fs1, fs2 = bstate["s1"], bstate["s2"]
            if fs2 is not None:
                b_epilogue(*fs2)
            fph, fpm, fpslab = fs1
            fres = b_denattn(fph, fpm, fpslab)
            c_tiles(2, range(0, 4), act_copy=True)
            b_epilogue(fph, fpm, *fres)

            # ---- Phase C remainder ----
            c_tiles(0, range(12, 16), pf=3, deep=True, act_copy=True)
            c_tiles(1, range(12, 16), pf=4, deep=True, act_copy=True)
            c_tiles(2, range(4, 16), pf=5, deep=True)
            for hc in range(3, 8):
                c_tiles(hc, range(16), pf=hc + 3 if hc + 3 <= 7 else None,
                        deep=True)

    return nc


_CACHE = {}


def build_program():
    if "nc" not in _CACHE:
        nc = bacc.Bacc()
        _emit(nc)
        nc.compile()
        _CACHE["nc"] = nc
    return _CACHE["nc"]


def host_inputs(positions, hidden_states, w_qkv, w_o):
    """Build the 8 per-core input maps (host-side shard + layout + bf16 cast)."""
    positions = np.asarray(positions)
    hidden_states = np.asarray(hidden_states, dtype=np.float32)
    w_qkv = np.asarray(w_qkv, dtype=np.float32)
    w_o = np.asarray(w_o, dtype=np.float32)

    inv_freq = 1.0 / (
        ROPE_THETA ** (np.arange(0, D, 2, dtype=np.float32) / D)
    )  # [64]
    trium = np.triu(np.ones((128, 128), dtype=np.float32)).astype(BF16)

    # per-batch tensors
    hTs, coss, sins = [], [], []
    for b in range(B):
        hT = (
            np.ascontiguousarray(hidden_states[b].T)  # [HIDDEN, S]
            .reshape(HIDDEN // 128, 128, S)
            .transpose(1, 0, 2)  # [128, ho, S]
        )
        hTs.append(np.ascontiguousarray(hT.astype(BF16)))
        ang = positions[b].astype(np.float32)[:, None] * inv_freq[None, :]  # [S,64]
        c = np.cos(ang).T  # [64, S]
        s = np.sin(ang).T
        coss.append(np.concatenate([c, c], axis=0).astype(BF16))
        sins.append(np.concatenate([s, s], axis=0).astype(BF16))

    in_maps = []
    for core in range(8):
        b, t = divmod(core, TP)
        qcols = w_qkv[:, t * QH * D : (t + 1) * QH * D]
        kcols = w_qkv[:, NH * D + t * KH * D : NH * D + (t + 1) * KH * D]
        vcols = w_qkv[:, (NH + NKV) * D + t * KH * D : (NH + NKV) * D + (t + 1) * KH * D]
        wshard = np.concatenate([qcols, kcols, vcols], axis=1)  # [4096, 1536]
        wq_t = (
            wshard.reshape(32, 128, NC_TILES, 128)
            .transpose(2, 1, 0, 3)  # [c, p, ho, m]
            .reshape(NC_TILES, 128, 32 * 128)
            .astype(BF16)
        )
        wo_shard = w_o[t * QH * D : (t + 1) * QH * D, :]  # [1024, 4096]
        wo_t = (
            wo_shard.reshape(QH, 128, 8, 512)
            .transpose(2, 1, 0, 3)  # [hc, p, co, n]
            .reshape(8, 128, 8 * 512)
            .astype(BF16)
        )
        in_maps.append(
            {
                "hT": hTs[b],
                "wq": np.ascontiguousarray(wq_t),
                "wo": np.ascontiguousarray(wo_t),
                "cosT": coss[b],
                "sinT": sins[b],
                "triuD": trium,
                "onesD": np.ones((1, 128), dtype=BF16),
                "onesMD": np.ones((128, 128), dtype=BF16),
            }
        )
    return in_maps


def gather_output(results):
    """Sum the 4 TP bf16 partials per batch in fp32 -> [B, S, HIDDEN]."""
    outs = []
    for b in range(B):
        acc = np.zeros((S, HIDDEN), dtype=np.float32)
        for t in range(TP):
            acc += results[b * TP + t]["out"].astype(np.float32)
        outs.append(acc)
    return np.stack(outs, axis=0)


def kernel(positions, hidden_states, w_qkv, w_o, trace=False):
    nc = build_program()
    in_maps = host_inputs(positions, hidden_states, w_qkv, w_o)
    last_err = None
    for attempt in range(3):
        try:
            res = bass_utils.run_bass_kernel_spmd(
                nc, in_maps, core_ids=list(range(8)), trace=trace
            )
            break
        except Exception as e:  # transient NRT/axon device errors
            last_err = e
            import time as _time

            _time.sleep(5 * (attempt + 1))
    else:
        raise last_err
    out = gather_output(res.results)
    if trace:
        kernel.last_exec_time_ns = res.exec_time_ns
        kernel.last_results = res
    return out


# revision 58
# speedup vs baseline: 1.2362x; 1.0711x over previous
"""Trainium2 Bass kernel for Mixtral-style attention (B=2, S=2048, 32 q / 8 kv heads, D=128).

Sharding: 2-way data parallel over batch x 4-way tensor parallel over heads
(8 cores). Each core computes QKV projection for its head shard, RoPE, causal
GQA attention, and a partial o_proj (row-sharded). Host sums the 4 bf16
partials per batch element in fp32.

All heavy matmuls run in bf16 with fp32 PSUM accumulation. Attention scores
are computed directly transposed (kT_blk^T @ qT_chunk) so exp(PSUM)->SBUF
lands straight in the probsT layout the attnT matmul needs; the causal mask
is a transposed-tril multiply on the diagonal 128x128 block only.

Softmax denominator: probsT blocks are group-summed on the DVE (bf16),
then ONE all-ones-stationary matmul per (head, chunk) turns the [128,512]
block-sum into the column-sum replicated across all 128 partitions. A fast
DVE reciprocal of that [128,512] tile feeds the attnT normalization multiply
directly -- no [1,512] row, no broadcast matmul.

Phase A (QKV projection, PE-bound) and phase B (attention, ACT/exp-heavy)
are software-interleaved per 512-token chunk level: while the PE chews
chunk m+1's projection matmuls, the ACT engine computes chunk m's exps and
the DVE its denominators, so no engine serializes the other. q chunks
rotate through a 2-slot buffer (chunk m is consumed by level m only).
Phase B itself keeps a 3-stage pipeline (scores(k) | den+attnV(k-1) |
epilogue(k-2)); diagonal-mask muls are emitted after the previous chunk's
den tree to avoid DVE head-of-line blocking.
"""

import os
import sys

import numpy as np

for _p in ("/opt/trn_rl_repo", "/root/.axon_site/_ro/trn_rl_repo"):
    if os.path.isdir(_p) and _p not in sys.path:
        sys.path.insert(0, _p)

import ml_dtypes  # noqa: E402

import concourse.bass as bass  # noqa: E402
import concourse.mybir as mybir  # noqa: E402
import concourse.tile as tile  # noqa: E402
from concourse import bacc, bass_utils  # noqa: E402

BF16 = ml_dtypes.bfloat16
F32 = mybir.dt.float32
BF = mybir.dt.bfloat16

B, S, HIDDEN = 2, 2048, 4096
NH, NKV, D = 32, 8, 128
TP, DP = 4, 2  # head-parallel x batch-parallel = 8 cores
QH = NH // TP  # 8 q heads per core
KH = NKV // TP  # 2 kv heads per core
NC_TILES = QH + 2 * KH  # 12 c-tiles of 128 per core (q..., k..., v...)
SC = 512  # s-chunk for phase A / attnT free dim
NSC = S // SC  # 4
NBLK = S // 128  # 16
ROPE_THETA = 10000.0
SM_SCALE = float(D) ** -0.5


def _interleave(a_list, b_list):
    """Merge two unit lists evenly (a spread across b)."""
    out = []
    ai = bi = 0
    na, nb = len(a_list), len(b_list)
    while ai < na or bi < nb:
        if bi >= nb or (ai < na and ai * nb <= bi * na):
            out.append(a_list[ai])
            ai += 1
        else:
            out.append(b_list[bi])
            bi += 1
    return out


def _emit(nc: bass.Bass):
    hT = nc.dram_tensor("hT", [128, HIDDEN // 128, S], BF, kind="ExternalInput")
    wq = nc.dram_tensor("wq", [NC_TILES, 128, 32 * 128], BF, kind="ExternalInput")
    wo = nc.dram_tensor("wo", [8, 128, 8 * 512], BF, kind="ExternalInput")
    cosT = nc.dram_tensor("cosT", [128, S], BF, kind="ExternalInput")
    sinT = nc.dram_tensor("sinT", [128, S], BF, kind="ExternalInput")
    triuD = nc.dram_tensor("triuD", [128, 128], BF, kind="ExternalInput")
    onesD = nc.dram_tensor("onesD", [1, 128], BF, kind="ExternalInput")
    onesMD = nc.dram_tensor("onesMD", [128, 128], BF, kind="ExternalInput")
    out = nc.dram_tensor("out", [S, HIDDEN], BF, kind="ExternalOutput")

    with tile.TileContext(nc) as tc:
        with (
            tc.tile_pool(name="const", bufs=1) as constp,
            tc.tile_pool(name="big", bufs=2) as bigp,
            tc.tile_pool(name="slab", bufs=2) as slabp,
            tc.tile_pool(name="wt", bufs=3) as wtp,
            tc.tile_pool(name="pers", bufs=1) as pers,
            tc.tile_pool(name="rope", bufs=1) as ropep,
            tc.tile_pool(name="acc", bufs=2) as accp,
            tc.tile_pool(name="rcp", bufs=1) as rcpp,
            tc.tile_pool(name="outp", bufs=4) as outp,
            tc.tile_pool(name="psum", bufs=2, space="PSUM") as psum,
            tc.tile_pool(name="psum_s", bufs=4, space="PSUM") as psum_s,
        ):
            # tiny consts first so the PE warm-up can start immediately
            triu = constp.tile([128, 128], BF, tag="triu")
            ones1 = constp.tile([1, 128], BF, tag="ones1")
            onesM = constp.tile([128, 128], BF, tag="onesM")
            nc.sync.dma_start(ones1, onesD[:])
            nc.sync.dma_start(triu, triuD[:])
            nc.sync.dma_start(onesM, onesMD[:])

            cos_sb = constp.tile([128, S], BF, tag="cos")
            sin_sb = constp.tile([128, S], BF, tag="sin")

            # persistent activations; q chunks rotate through 2 slots
            qT = pers.tile([128, QH, 2, SC], BF, tag="qT")  # [d, head, slot, s]
            kT = pers.tile([128, KH, S], BF, tag="kT")
            vN = pers.tile([128, KH * NBLK, 128], BF, tag="vN")  # [sk, kv*blk, d]
            aT = pers.tile([128, QH, S], BF, tag="aT")  # [d, head, s]

            def rope_into(dst, ps, sc):
                # dst = ps * cos + rot(ps) * sin ; rot = [-x2, x1]
                rot = ropep.tile([128, SC], F32, tag="rot")
                nc.scalar.mul(rot[0:64, :], ps[64:128, :], -1.0)
                nc.scalar.copy(rot[64:128, :], ps[0:64, :])
                t2 = ropep.tile([128, SC], F32, tag="t2")
                cs = cos_sb[:, sc * SC : (sc + 1) * SC]
                sn = sin_sb[:, sc * SC : (sc + 1) * SC]
                nc.vector.tensor_mul(t2, ps, cs)
                nc.vector.tensor_mul(rot, rot, sn)
                nc.vector.tensor_add(dst, t2, rot)

            # ---- Phase A units: one c-tile of QKV^T = w^T @ h^T + RoPE ----
            def emit_hTc(sc, parts=16):
                t = bigp.tile([128, 32, SC], BF, tag="hTc")
                w = 32 // parts
                for hq in range(parts):
                    nc.sync.dma_start(
                        t[:, hq * w : (hq + 1) * w, :],
                        hT[:, hq * w : (hq + 1) * w, sc * SC : (sc + 1) * SC],
                    )
                return t

            def emit_wct(c, parts=8):
                wct = wtp.tile([128, 32 * 128], BF, tag="wt")
                w = 4096 // parts
                for hq in range(parts):
                    nc.sync.dma_start(
                        wct[:, hq * w : (hq + 1) * w],
                        wq[c, :, hq * w : (hq + 1) * w],
                    )
                return wct

            def a_unit(sc, c, hTc, wct=None):
                if wct is None:
                    wct = emit_wct(c)
                ps = psum.tile([128, SC], F32, tag="mm512")
                for ho in range(32):
                    nc.tensor.matmul(
                        ps,
                        wct[:, ho * 128 : (ho + 1) * 128],
                        hTc[:, ho, :],
                        start=(ho == 0),
                        stop=(ho == 31),
                        skip_group_check=True,
                    )
                if c < QH:
                    rope_into(qT[:, c, sc % 2, :], ps, sc)
                elif c < QH + KH:
                    rope_into(kT[:, c - QH, sc * SC : (sc + 1) * SC], ps, sc)
                else:
                    kv = c - QH - KH
                    vt = ropep.tile([128, SC], BF, tag="vt")
                    nc.scalar.copy(vt, ps)
                    for j in range(SC // 128):
                        blk = sc * 4 + j
                        nc.sync.dma_start(
                            vN[:, kv * NBLK + blk, :],
                            vt[:, j * 128 : (j + 1) * 128],
                            transpose=True,
                        )

            # ---- Phase B units ----
            # slab[:, j, :] holds (unnormalized) probsT for sk-block j of the
            # current sq-chunk: all scores are computed directly transposed
            # (kT_blk^T @ qT_chunk) + exp from PSUM. Diagonal rows only cover
            # their causal sq columns; the diagonal 128x128 block gets a
            # transposed-tril (triu) mask applied post-exp (emitted late, see
            # b_unit).
            def b_scores(h, m):
                kv = h // (QH // KH)
                slab = slabp.tile([128, NBLK, SC], BF, tag="slab")
                qm = qT[:, h, m % 2, :]
                for j in range(4 * m + 4):
                    jj = j - 4 * m  # >= 0 for diagonal-region rows
                    c0 = max(0, jj) * 128
                    sps = psum_s.tile([128, 512], F32, tag="scores")
                    nc.tensor.matmul(
                        sps[:, : 512 - c0],
                        kT[:, kv, j * 128 : (j + 1) * 128],
                        qm[:, c0:],
                        start=True,
                        stop=True,
                        skip_group_check=True,
                    )
                    nc.scalar.activation(
                        slab[:, j, c0:],
                        sps[:, : 512 - c0],
                        mybir.ActivationFunctionType.Exp,
                        scale=SM_SCALE,
                    )
                return slab

            def b_mask(m, slab):
                for jj in range(4):
                    j = 4 * m + jj
                    c0 = jj * 128
                    blk = slab[:, j, c0 : c0 + 128]
                    nc.vector.tensor_mul(blk, blk, triu)

            def b_denattn(h, m, slab):
                kv = h // (QH // KH)
                # DVE block-sum of slab -> acc[:, 0, :]. Diagonal group (last
                # four j blocks) is width-aware in slot 1; full blocks chain
                # into slot 0. All bf16 SBUF (fast DVE mode).
                acc = accp.tile([128, 2, 512], BF, tag="acc")
                sd = 1 if m > 0 else 0
                nc.vector.tensor_copy(acc[:, sd, :], slab[:, 4 * m, :])
                for jj in range(1, 4):
                    c0 = jj * 128
                    nc.vector.tensor_add(
                        acc[:, sd, c0:], acc[:, sd, c0:], slab[:, 4 * m + jj, c0:]
                    )
                if m > 0:
                    nc.vector.tensor_add(acc[:, 0, :], slab[:, 0, :], slab[:, 1, :])
                    for j in range(2, 4 * m):
                        nc.vector.tensor_add(acc[:, 0, :], acc[:, 0, :], slab[:, j, :])
                    nc.vector.tensor_add(acc[:, 0, :], acc[:, 0, :], acc[:, 1, :])

                # attnV: accumulate over all j blocks
                aps = psum.tile([128, 512], F32, tag="attn")
                for j in range(4 * m):
                    nc.tensor.matmul(
                        aps, vN[:, kv * NBLK + j, :], slab[:, j, :],
                        start=(j == 0), stop=False, skip_group_check=True,
                    )
                for jj in range(4):
                    j = 4 * m + jj
                    cs = slice(jj * 128, 512)
                    first = m == 0 and jj == 0
                    nc.tensor.matmul(
                        aps[:, cs], vN[:, kv * NBLK + j, :], slab[:, j, cs],
                        start=first, stop=(jj == 3), skip_group_check=True,
                    )
                # den replicated to all partitions: ones[128,128]^T @ acc0
                den = psum.tile([128, 512], F32, tag="mm512")
                nc.tensor.matmul(
                    den, onesM, acc[:, 0, :], start=True, stop=True,
                    skip_group_check=True,
                )
                rcp = rcpp.tile([128, 512], F32, tag="rcp")
                nc.vector.reciprocal_approx_fast(rcp, den)
                return aps, rcp

            def b_epilogue(h, m, aps, rcp):
                nc.vector.tensor_mul(aT[:, h, m * 512 : (m + 1) * 512], aps, rcp)

            # 3-stage pipeline state over B units (global across levels)
            bstate = {"s1": None, "s2": None}

            def b_unit(h, m):
                slab = b_scores(h, m)
                s1, s2 = bstate["s1"], bstate["s2"]
                # epilogue(k-2) BEFORE denattn(k-1): keeps the single rcp
                # buffer strictly read-then-write in DVE program order
                if s1 is not None and s2 is not None:
                    b_epilogue(*s2)
                if s1 is not None:
                    ph, pm, pslab = s1
                    bstate["s2"] = (ph, pm) + b_denattn(ph, pm, pslab)
                b_mask(m, slab)  # after prev chunk's den tree (DVE order)
                bstate["s1"] = (h, m, slab)

            def b_flush():
                s1, s2 = bstate["s1"], bstate["s2"]
                if s2 is not None:
                    b_epilogue(*s2)
                ph, pm, pslab = s1
                b_epilogue(ph, pm, *b_denattn(ph, pm, pslab))

            # ---- Prologue: first c-tile inputs in flight during warm-up ----
            # ultra-fine splits so the first chunks land on idle queues fast;
            # weights first -- the first LDWEIGHTS is the startup blocker
            wct0 = emit_wct(0, parts=16)
            hcur = emit_hTc(0, parts=32)

            # HAM warm-up: ~10us of dummy matmuls on the tiny constants
            # while the first hidden/weight DMAs are in flight, so the PE
            # is already un-throttled (K=8/8) when real data arrives.
            wps = psum_s.tile([128, 512], F32, tag="scores")
            for w in range(48):
                nc.tensor.matmul(
                    wps[:, :128], ones1, ones1, start=(w == 0), stop=(w == 47),
                    skip_group_check=True,
                )
            dwarm = rcpp.tile([128, 512], F32, tag="rcp")
            nc.vector.tensor_copy(dwarm[:, :128], wps[:, :128])

            nc.sync.dma_start(cos_sb, cosT[:])
            nc.sync.dma_start(sin_sb, sinT[:])
            a_unit(0, 0, hcur, wct0)
            for c in range(1, NC_TILES):
                a_unit(0, c, hcur)
                if c == 9:
                    b_unit(0, 0)  # spread early exps into the prologue
                elif c == 10:
                    b_unit(1, 0)

            # ---- Phase C helpers: o_proj st-tiles for one w_o head-tile ----
            wo_tiles = {}

            def c_prefetch(hc):
                wot = wtp.tile([128, 8 * 512], BF, tag="wt")
                for hq in range(8):
                    nc.sync.dma_start(
                        wot[:, hq * 512 : (hq + 1) * 512],
                        wo[hc, :, hq * 512 : (hq + 1) * 512],
                    )
                wo_tiles[hc] = wot

            def c_tiles(hc, sts, pf=None, deep=False):
                wot = wo_tiles[hc]
                for sti, st in enumerate(sts):
                    if pf is not None and sti == len(sts) // 2:
                        c_prefetch(pf)
                    # after phase B both psum pools are free: alternate tags
                    # for a 4-deep rotation so the DVE cast is never waited on
                    tag = ("attn" if sti % 2 else "mm512") if deep else "mm512"
                    ops = psum.tile([128, 512], F32, tag=tag)
                    for cb in range(QH):
                        nc.tensor.matmul(
                            ops,
                            aT[:, cb, st * 128 : (st + 1) * 128],
                            wot[:, cb * 512 : (cb + 1) * 512],
                            start=(cb == 0),
                            stop=(cb == QH - 1),
                            skip_group_check=True,
                        )
                    ot = outp.tile([128, 512], BF, tag="ot")
                    nc.vector.tensor_copy(ot, ops)
                    for half in range(2):
                        nc.sync.dma_start(
                            out[
                                st * 128 : (st + 1) * 128,
                                hc * 512 + half * 256 : hc * 512 + (half + 1) * 256,
                            ],
                            ot[:, half * 256 : (half + 1) * 256],
                        )

            # ---- Levels: A(m+1) interleaved with B(*, m); level 3 weaves
            # in early o_proj tiles for hc=0,1 (aT chunks 0..2 are ready) ----
            for m in range(NSC):
                h0 = 2 if m == 0 else 0  # first two m=0 units are in prologue
                bunits = [(lambda h=h, m=m: b_unit(h, m)) for h in range(h0, QH)]
                if m < NSC - 1:
                    hnext = emit_hTc(m + 1)
                    aunits = [
                        (lambda sc=m + 1, c=c, t=hnext: a_unit(sc, c, t))
                        for c in range(NC_TILES)
                    ]
                    # lead with B units so the fresh hTc DMA gets headroom
                    lead = 2 if m == 0 else 1
                    merged = bunits[:lead] + _interleave(aunits, bunits[lead:])
                else:
                    c_prefetch(0)
                    c_prefetch(1)
                    fillers = [
                        (lambda hc=hc, g=g: c_tiles(hc, range(4 * g, 4 * g + 4)))
                        for g in range(3)
                        for hc in (0, 1)
                    ]
                    merged = bunits[:2] + _interleave(fillers, bunits[2:])
                for u in merged:
                    u()
            c_prefetch(2)
            # flush with PE cover: independent o_proj tiles run while the
            # final den/recip/epilogue DVE chain drains
            fs1, fs2 = bstate["s1"], bstate["s2"]
            if fs2 is not None:
                b_epilogue(*fs2)
            fph, fpm, fpslab = fs1
            fres = b_denattn(fph, fpm, fpslab)
            c_tiles(2, range(0, 4))
            b_epilogue(fph, fpm, *fres)

            # ---- Phase C remainder ----
            c_tiles(0, range(12, 16), pf=3, deep=True)
            c_tiles(1, range(12, 16), pf=4, deep=True)
            c_tiles(2, range(4, 16), pf=5, deep=True)
            for hc in range(3, 8):
                c_tiles(hc, range(16), pf=hc + 3 if hc + 3 <= 7 else None,
                        deep=True)

    return nc


_CACHE = {}


def build_program():
    if "nc" not in _CACHE:
        nc = bacc.Bacc()
        _emit(nc)
        nc.compile()
        _CACHE["nc"] = nc
    return _CACHE["nc"]


def host_inputs(positions, hidden_states, w_qkv, w_o):
    """Build the 8 per-core input maps (host-side shard + layout + bf16 cast)."""
    positions = np.asarray(positions)
    hidden_states = np.asarray(hidden_states, dtype=np.float32)
    w_qkv = np.asarray(w_qkv, dtype=np.float32)
    w_o = np.asarray(w_o, dtype=np.float32)

    inv_freq = 1.0 / (
        ROPE_THETA ** (np.arange(0, D, 2, dtype=np.float32) / D)
    )  # [64]
    trium = np.triu(np.ones((128, 128), dtype=np.float32)).astype(BF16)

    # per-batch tensors
    hTs, coss, sins = [], [], []
    for b in range(B):
        hT = (
            np.ascontiguousarray(hidden_states[b].T)  # [HIDDEN, S]
            .reshape(HIDDEN // 128, 128, S)
            .transpose(1, 0, 2)  # [128, ho, S]
        )
        hTs.append(np.ascontiguousarray(hT.astype(BF16)))
        ang = positions[b].astype(np.float32)[:, None] * inv_freq[None, :]  # [S,64]
        c = np.cos(ang).T  # [64, S]
        s = np.sin(ang).T
        coss.append(np.concatenate([c, c], axis=0).astype(BF16))
        sins.append(np.concatenate([s, s], axis=0).astype(BF16))

    in_maps = []
    for core in range(8):
        b, t = divmod(core, TP)
        qcols = w_qkv[:, t * QH * D : (t + 1) * QH * D]
        kcols = w_qkv[:, NH * D + t * KH * D : NH * D + (t + 1) * KH * D]
        vcols = w_qkv[:, (NH + NKV) * D + t * KH * D : (NH + NKV) * D + (t + 1) * KH * D]
        wshard = np.concatenate([qcols, kcols, vcols], axis=1)  # [4096, 1536]
        wq_t = (
            wshard.reshape(32, 128, NC_TILES, 128)
            .transpose(2, 1, 0, 3)  # [c, p, ho, m]
            .reshape(NC_TILES, 128, 32 * 128)
            .astype(BF16)
        )
        wo_shard = w_o[t * QH * D : (t + 1) * QH * D, :]  # [1024, 4096]
        wo_t = (
            wo_shard.reshape(QH, 128, 8, 512)
            .transpose(2, 1, 0, 3)  # [hc, p, co, n]
            .reshape(8, 128, 8 * 512)
            .astype(BF16)
        )
        in_maps.append(
            {
                "hT": hTs[b],
                "wq": np.ascontiguousarray(wq_t),
                "wo": np.ascontiguousarray(wo_t),
                "cosT": coss[b],
                "sinT": sins[b],
                "triuD": trium,
                "onesD": np.ones((1, 128), dtype=BF16),
                "onesMD": np.ones((128, 128), dtype=BF16),
            }
        )
    return in_maps


def gather_output(results):
    """Sum the 4 TP bf16 partials per batch in fp32 -> [B, S, HIDDEN]."""
    outs = []
    for b in range(B):
        acc = np.zeros((S, HIDDEN), dtype=np.float32)
        for t in range(TP):
            acc += results[b * TP + t]["out"].astype(np.float32)
        outs.append(acc)
    return np.stack(outs, axis=0)


def kernel(positions, hidden_states, w_qkv, w_o, trace=False):
    nc = build_program()
    in_maps = host_inputs(positions, hidden_states, w_qkv, w_o)
    last_err = None
    for attempt in range(3):
        try:
            res = bass_utils.run_bass_kernel_spmd(
                nc, in_maps, core_ids=list(range(8)), trace=trace
            )
            break
        except Exception as e:  # transient NRT/axon device errors
            last_err = e
            import time as _time

            _time.sleep(5 * (attempt + 1))
    else:
        raise last_err
    out = gather_output(res.results)
    if trace:
        kernel.last_exec_time_ns = res.exec_time_ns
        kernel.last_results = res
    return out
